# revision 1
# baseline (speedup 1.0000x reference)
"""Trainium2 Bass kernel for the ConductanceLIFNetwork problem.

Strategy: column-shard the 1536 postsynaptic neurons across 8 cores (192
each), batch (32) replicated.  Per timestep each core computes its slice of
the recurrent conductance inputs with the spike vector as the PE stationary
operand (12 accumulating matmuls streaming 384 weight columns), runs the
fused LIF state updates on DVE/Pool, transposes its new spike slice on the
PE, and exchanges slices with the other cores through an AllGather so every
core has the full presynaptic spike vector for the next step.  The
feedforward input matmuls depend only on the (known) input spikes, so they
are issued first each step and execute while the AllGather is in flight.
"""

import math

import numpy as np

# ---- problem constants (hardcoded; kernel.py must be self-contained) ----
N_NEURONS = 1536
N_INPUTS = 768
BATCH = 32
T_STEPS = 256
N_CORES = 8
COLS = N_NEURONS // N_CORES  # 192 postsynaptic neurons per core
DT = 1.0

CELL_TAU_MEM = np.array([20.0, 10.0], np.float32)
CELL_TAUREF = np.array([2.0, 1.0], np.float32)
# theta=-50, u_reset=e_l=-65, g_l=10 for both cell types
SYN_TAU_RISE = np.array([0.5, 2.0, 0.5], np.float32)
SYN_TAU_DECAY = np.array([2.0, 100.0, 5.0], np.float32)

AR = [float(math.exp(-DT / t)) for t in SYN_TAU_RISE]   # x rise decays
AD = [float(math.exp(-DT / t)) for t in SYN_TAU_DECAY]  # g decay
ARF = float(math.exp(-DT / 0.5))
ADF = float(math.exp(-DT / 2.0))

K_REC = N_NEURONS // 128   # 12 contraction tiles for recurrent matmul
K_FF = N_INPUTS // 128     # 6 contraction tiles for feedforward matmul


def _build(T: int):
    import os
    abl = set(os.environ.get("KABL", "").split(","))
    import concourse.bacc as bacc
    import concourse.tile as tile
    import concourse.mybir as mybir

    f32 = mybir.dt.float32
    op = mybir.AluOpType

    nc = bacc.Bacc(
        "TRN2",
        target_bir_lowering=False,
        debug=False,
        enable_asserts=False,
        num_devices=N_CORES,
    )

    # ---- kernel I/O ----
    w_in = nc.dram_tensor("w_in", [K_REC, 128, 2 * COLS], f32, kind="ExternalInput").ap()
    wf_in = nc.dram_tensor("wf_in", [K_FF, 128, COLS], f32, kind="ExternalInput").ap()
    itT_in = nc.dram_tensor("itT_in", [K_FF, 128, T, BATCH], f32, kind="ExternalInput").ap()
    lc_in = nc.dram_tensor("lc_in", [BATCH, COLS], f32, kind="ExternalInput").ap()
    rs_in = nc.dram_tensor("rs_in", [BATCH, COLS], f32, kind="ExternalInput").ap()
    id_in = nc.dram_tensor("id_in", [BATCH, BATCH], f32, kind="ExternalInput").ap()
    out_s = nc.dram_tensor("out_s", [T, BATCH, COLS], f32, kind="ExternalOutput").ap()
    out_u = nc.dram_tensor("out_u", [T, BATCH, COLS], f32, kind="ExternalOutput").ap()

    with tile.TileContext(nc) as tc:
        with (
            tc.tile_pool(name="const", bufs=1) as cpool,
            tc.tile_pool(name="state", bufs=1) as spool,
            tc.tile_pool(name="st", bufs=2) as st_pool,
            tc.tile_pool(name="itt", bufs=4) as it_pool,
            tc.tile_pool(name="pin", bufs=2, space="PSUM") as pin_pool,
            tc.tile_pool(name="pff", bufs=2, space="PSUM") as pff_pool,
            tc.tile_pool(name="ptr", bufs=2, space="PSUM") as ptr_pool,
            tc.tile_pool(name="agi", bufs=2, space="DRAM") as agi_pool,
            tc.tile_pool(name="ago", bufs=2, space="DRAM") as ago_pool,
        ):
            # ---- load constants ----
            w_sb = cpool.tile([128, K_REC, 2 * COLS], f32)
            nc.sync.dma_start(w_sb[:], w_in.rearrange("k p c -> p k c"))
            wf_sb = cpool.tile([128, K_FF, COLS], f32)
            nc.sync.dma_start(wf_sb[:], wf_in.rearrange("k p c -> p k c"))
            lc_t = cpool.tile([BATCH, COLS], f32)
            nc.sync.dma_start(lc_t[:], lc_in)
            rs_t = cpool.tile([BATCH, COLS], f32)
            nc.sync.dma_start(rs_t[:], rs_in)
            ident = cpool.tile([BATCH, BATCH], f32)
            nc.sync.dma_start(ident[:], id_in)
            neg65 = cpool.tile([BATCH, COLS], f32)
            nc.vector.memset(neg65[:], -65.0)

            # ---- persistent state tiles ----
            def state(val=0.0):
                t_ = spool.tile([BATCH, COLS], f32, tag=f"st{state.i}")
                state.i += 1
                nc.vector.memset(t_[:], val)
                return t_
            state.i = 0

            U = state(-65.0)
            ref = state()
            x0, x1, x2 = state(), state(), state()
            g0, g1, g2 = state(), state(), state()
            xF, gF = state(), state()
            s_sb = state()
            m_t = state()
            tt_ = state()
            isyn = state()
            inner = state()

            sT_cur = st_pool.tile([128, K_REC, BATCH], f32)
            nc.vector.memset(sT_cur[:], 0.0)

            stt = nc.vector.scalar_tensor_tensor
            stt_g = nc.vector.scalar_tensor_tensor

            for t in range(T):
                # FF matmul first: no dependence on the gathered spikes, so the
                # PE can chew on it while the previous step's AllGather lands.
                itT = it_pool.tile([128, K_FF, BATCH], f32)
                nc.sync.dma_start(itT[:], itT_in[:, :, t, :].rearrange("k p b -> p k b"))
                pff = pff_pool.tile([BATCH, COLS], f32)
                for k in range(K_FF):
                    nc.tensor.matmul(pff[:], itT[:, k, :], wf_sb[:, k, :],
                                     start=(k == 0), stop=(k == K_FF - 1))

                pinp = pin_pool.tile([BATCH, 2 * COLS], f32)
                if "nomm" in abl:
                    nc.vector.memset(pinp[:], 0.0)
                for k in range(0 if "nomm" in abl else K_REC):
                    nc.tensor.matmul(pinp[:], sT_cur[:, k, :], w_sb[:, k, :],
                                     start=(k == 0), stop=(k == K_REC - 1))

                # refractory bookkeeping from previous step's state (no dep on
                # this step's matmul) — runs on Pool during the matmuls.
                nc.gpsimd.tensor_scalar(m_t[:], ref[:], 0.0, None, op0=op.is_gt)
                nc.gpsimd.tensor_scalar(ref[:], ref[:], -1.0, 0.0, op0=op.add, op1=op.max)

                # FF dual-exponential states
                stt(xF[:], xF[:], ARF, pff[:], op.mult, op.add)
                stt_g(gF[:], gF[:], ADF, xF[:], op.mult, op.add)

                # recurrent dual-exponential states
                stt(x0[:], x0[:], AR[0], pinp[:, 0:COLS], op.mult, op.add)
                stt(x1[:], x1[:], AR[1], pinp[:, 0:COLS], op.mult, op.add)
                stt(x2[:], x2[:], AR[2], pinp[:, COLS:2 * COLS], op.mult, op.add)
                stt_g(g0[:], g0[:], AD[0], x0[:], op.mult, op.add)
                stt_g(g1[:], g1[:], AD[1], x1[:], op.mult, op.add)
                stt(g2[:], g2[:], AD[2], x2[:], op.mult, op.add)

                # gtot = g0 + 0.5*g1 + g2 + gF   (gbar = [1, .5, 1], FF_GBAR=1)
                stt(tt_[:], g1[:], 0.5, g0[:], op.mult, op.add)
                stt_g(tt_[:], g2[:], 1.0, tt_[:], op.mult, op.add)
                stt(tt_[:], gF[:], 1.0, tt_[:], op.mult, op.add)
                # I_syn = -70*g2 - gtot*U   (gbarE = [0, 0, -70], FF_EREV=0)
                nc.vector.tensor_tensor(inner[:], tt_[:], U[:], op.mult)
                stt(isyn[:], g2[:], -70.0, inner[:], op.mult, op.subtract)
                # U += lc * (10*(-65-U) + I_syn) = lc * ((-10*U + I_syn) - 650)
                stt(inner[:], U[:], -10.0, isyn[:], op.mult, op.add)
                nc.vector.tensor_scalar(inner[:], inner[:], -650.0, None, op0=op.add)
                nc.vector.tensor_tensor(inner[:], inner[:], lc_t[:], op.mult)
                nc.vector.tensor_tensor(U[:], U[:], inner[:], op.add)
                # refractory clamp, spike, reset
                nc.vector.copy_predicated(U[:], m_t[:].bitcast(mybir.dt.int32), neg65[:])
                nc.vector.tensor_scalar(s_sb[:], U[:], -50.0, None, op0=op.is_ge)
                s_mask = s_sb[:].bitcast(mybir.dt.int32)
                nc.vector.copy_predicated(U[:], s_mask, neg65[:])
                nc.vector.copy_predicated(ref[:], s_mask, rs_t[:])

                if t < T - 1:
                    # transpose own spike slice to [neuron, batch] and gather
                    ptr = ptr_pool.tile([128, 2 * BATCH], f32)
                    nc.tensor.transpose(ptr[0:128, 0:BATCH], s_sb[:, 0:128], ident[:])
                    nc.tensor.transpose(ptr[0:64, BATCH:2 * BATCH],
                                        s_sb[:, 128:COLS], ident[:])
                    sp_st = st_pool.tile([128, 2 * BATCH], f32, tag="spst")
                    nc.scalar.copy(sp_st[:], ptr[:])
                    agi = agi_pool.tile([COLS, BATCH], f32)
                    nc.sync.dma_start(agi[0:128, :], sp_st[0:128, 0:BATCH])
                    nc.sync.dma_start(agi[128:COLS, :], sp_st[0:64, BATCH:2 * BATCH])
                    ago = ago_pool.tile([N_NEURONS, BATCH], f32)
                    if "nocc" in abl:
                        nc.sync.dma_start(ago.opt()[0:COLS], agi.opt())
                    else:
                        nc.gpsimd.collective_compute(
                            "AllGather",
                            op.bypass,
                            replica_groups=[list(range(N_CORES))],
                            ins=[agi.opt()],
                            outs=[ago.opt()],
                        )
                    sT_cur = st_pool.tile([128, K_REC, BATCH], f32)
                    ago_v = ago.opt().rearrange("(k p) b -> p k b", p=128)
                    # 12 separate DMAs spread across HWDGE queues: each moves a
                    # contiguous 16KB k-tile, cutting the serial gather-return
                    # latency vs one strided transfer.
                    if "onedma" in abl:
                        nc.sync.dma_start(sT_cur[:], ago_v)
                    else:
                        for k in range(K_REC):
                            nc.sync.dma_start(sT_cur[:, k, :], ago_v[:, k, :])

                if "nodma" not in abl:
                    nc.sync.dma_start(out_s[t], s_sb[:])
                    nc.sync.dma_start(out_u[t], U[:])

    nc.compile()
    return nc


def _prep_inputs(input_spikes, weights, weights_FF, scaling_factors,
                 scaling_factors_FF, cell_type_indices, cell_type_indices_FF, T):
    ct = np.asarray(cell_type_indices).astype(np.int64)
    sf = np.asarray(scaling_factors, np.float32)[ct[:, None], ct[None, :]]
    W = np.asarray(weights, np.float32) * sf
    mask_e = (ct == 0).astype(np.float32)[:, None]
    W_e = W * mask_e
    W_i = W * (1.0 - mask_e)
    ctF = np.asarray(cell_type_indices_FF).astype(np.int64)
    sfF = np.asarray(scaling_factors_FF, np.float32)[ctF[:, None], ct[None, :]]
    WF = np.asarray(weights_FF, np.float32) * sfF

    tau_mem = CELL_TAU_MEM[ct]
    lc = (DT / (tau_mem * 10.0)).astype(np.float32)        # leak_coef per neuron
    rs = (CELL_TAUREF[ct] / DT).astype(np.float32)          # refractory steps

    isp = np.ascontiguousarray(np.asarray(input_spikes, np.float32)[:, :T, :])
    # itT[k, p, t, b] = input_spikes[b, t, 128k+p]
    itT = np.ascontiguousarray(
        isp.transpose(2, 1, 0).reshape(K_FF, 128, T, BATCH))

    ident = np.eye(BATCH, dtype=np.float32)

    in_maps = []
    for c in range(N_CORES):
        cols = slice(c * COLS, (c + 1) * COLS)
        wcat = np.concatenate([W_e[:, cols], W_i[:, cols]], axis=1)  # (1536, 384)
        w_in = np.ascontiguousarray(wcat.reshape(K_REC, 128, 2 * COLS))
        wf_c = np.ascontiguousarray(WF[:, cols].reshape(K_FF, 128, COLS))
        lc_c = np.broadcast_to(lc[cols], (BATCH, COLS)).copy()
        rs_c = np.broadcast_to(rs[cols], (BATCH, COLS)).copy()
        in_maps.append({
            "w_in": w_in,
            "wf_in": wf_c,
            "itT_in": itT,
            "lc_in": lc_c,
            "rs_in": rs_c,
            "id_in": ident,
        })
    return in_maps


_NC_CACHE = {}


def run(inputs: dict, T: int = T_STEPS, trace: bool = False):
    from concourse.bass_utils import run_bass_kernel_spmd

    if T not in _NC_CACHE:
        _NC_CACHE[T] = _build(T)
    nc = _NC_CACHE[T]
    in_maps = _prep_inputs(T=T, **inputs)
    res = run_bass_kernel_spmd(
        nc, in_maps, core_ids=list(range(N_CORES)), trace=trace,
    )
    spk = np.concatenate([r["out_s"] for r in res.results], axis=2)
    volts = np.concatenate([r["out_u"] for r in res.results], axis=2)
    spk = np.ascontiguousarray(spk.transpose(1, 0, 2))
    volts = np.ascontiguousarray(volts.transpose(1, 0, 2))
    return (spk, volts), res


def kernel(**inputs):
    (spk, volts), _ = run(inputs, T=T_STEPS, trace=False)
    return spk, volts



# revision 7
# speedup vs baseline: 3.4708x; 3.4708x over previous
"""Trainium2 Bass kernel for the ConductanceLIFNetwork problem.

Strategy: column-shard the 1536 postsynaptic neurons across 8 cores (192
each), batch (32) replicated.  The feedforward conductance trace gF is
input-independent of the recurrent state, so it is precomputed entirely
before the serial loop: input spikes arrive bitpacked (32 bits -> u8x4 per
batch row), are expanded to bf16 on the vector engine, projected through
the bf16 FF weights on the PE, and run through the dual-exponential scan,
leaving Gp[t] = gF[t] + 10 resident in SBUF.  The per-timestep loop then
only computes the recurrent conductance inputs (12 accumulating bf16
matmuls with the gathered spike vector stationary), the fused LIF state
update on DVE/Pool, a PE transpose of the new spike slice, and an
AllGather so every core has the full presynaptic spike vector for the
next step.  Outputs are staged in SBUF and flushed every 16 steps as u8
spikes and fp8(U+65) voltages to minimize host I/O.
"""

import math

import numpy as np

# ---- problem constants (hardcoded; kernel.py must be self-contained) ----
N_NEURONS = 1536
N_INPUTS = 768
BATCH = 32
T_STEPS = 256
N_CORES = 8
COLS = N_NEURONS // N_CORES  # 192 postsynaptic neurons per core
DT = 1.0

CELL_TAU_MEM = np.array([20.0, 10.0], np.float32)
CELL_TAUREF = np.array([2.0, 1.0], np.float32)
# theta=-50, u_reset=e_l=-65, g_l=10 for both cell types
SYN_TAU_RISE = np.array([0.5, 2.0, 0.5], np.float32)
SYN_TAU_DECAY = np.array([2.0, 100.0, 5.0], np.float32)

AR = [float(math.exp(-DT / t)) for t in SYN_TAU_RISE]   # x rise decays
AD = [float(math.exp(-DT / t)) for t in SYN_TAU_DECAY]  # g decay
ARF = float(math.exp(-DT / 0.5))
ADF = float(math.exp(-DT / 2.0))

K_REC = N_NEURONS // 128   # 12 contraction tiles for recurrent matmul
K_FF = N_INPUTS // 128     # 6 contraction tiles for feedforward matmul
TBLK = 64                  # t-chunk for the spike unpack buffer
OBLK = 16                  # output staging block


def _build(T: int):
    import os
    abl = set(os.environ.get("KABL", "").split(","))
    import concourse.bacc as bacc
    import concourse.tile as tile
    import concourse.mybir as mybir

    f32 = mybir.dt.float32
    bf16 = mybir.dt.bfloat16
    u8 = mybir.dt.uint8
    f8 = mybir.dt.float8e4
    op = mybir.AluOpType

    nc = bacc.Bacc(
        "TRN2",
        target_bir_lowering=False,
        debug=False,
        enable_asserts=False,
        num_devices=N_CORES,
    )

    # ---- kernel I/O ----
    w_in = nc.dram_tensor("w_in", [K_REC, 128, 2 * COLS], bf16, kind="ExternalInput").ap()
    wf_in = nc.dram_tensor("wf_in", [K_FF, 128, COLS], bf16, kind="ExternalInput").ap()
    pk_in = nc.dram_tensor("pk_in", [K_FF, 128, T, BATCH // 8], u8, kind="ExternalInput").ap()
    lc_in = nc.dram_tensor("lc_in", [BATCH, COLS], f32, kind="ExternalInput").ap()
    rs_in = nc.dram_tensor("rs_in", [BATCH, COLS], f32, kind="ExternalInput").ap()
    id_in = nc.dram_tensor("id_in", [BATCH, BATCH], f32, kind="ExternalInput").ap()
    out_s = nc.dram_tensor("out_s", [BATCH, T, COLS], u8, kind="ExternalOutput").ap()
    out_u = nc.dram_tensor("out_u", [BATCH, T, COLS], f8, kind="ExternalOutput").ap()

    n_ob = T // OBLK

    with tile.TileContext(nc) as tc:
        with (
            tc.tile_pool(name="const", bufs=1) as cpool,
            tc.tile_pool(name="state", bufs=1) as spool,
            tc.tile_pool(name="unp", bufs=2) as upool,
            tc.tile_pool(name="st", bufs=2) as st_pool,
            tc.tile_pool(name="stage", bufs=2) as o_pool,
            tc.tile_pool(name="pin", bufs=2, space="PSUM") as pin_pool,
            tc.tile_pool(name="pff", bufs=4, space="PSUM") as pff_pool,
            tc.tile_pool(name="ptr", bufs=2, space="PSUM") as ptr_pool,
            tc.tile_pool(name="agi", bufs=2, space="DRAM") as agi_pool,
            tc.tile_pool(name="ago", bufs=2, space="DRAM") as ago_pool,
        ):
            # ---- load constants ----
            w_sb = cpool.tile([128, K_REC, 2 * COLS], bf16)
            nc.sync.dma_start(w_sb[:], w_in.rearrange("k p c -> p k c"))
            wf_sb = cpool.tile([128, K_FF, COLS], bf16)
            nc.sync.dma_start(wf_sb[:], wf_in.rearrange("k p c -> p k c"))
            pk_sb = cpool.tile([128, K_FF, T, BATCH // 8], u8)
            nc.sync.dma_start(pk_sb[:], pk_in.rearrange("k p t w -> p k t w"))
            lc_t = cpool.tile([BATCH, COLS], f32)
            nc.sync.dma_start(lc_t[:], lc_in)
            rs_t = cpool.tile([BATCH, COLS], f32)
            nc.sync.dma_start(rs_t[:], rs_in)
            ident = cpool.tile([BATCH, BATCH], f32)
            nc.sync.dma_start(ident[:], id_in)
            neg65 = cpool.tile([BATCH, COLS], f32)
            nc.vector.memset(neg65[:], -65.0)
            c650 = cpool.tile([BATCH, COLS], f32)
            nc.vector.memset(c650[:], 650.0)

            # Gp[t] = gF[t] + 10 (per-neuron feedforward conductance + leak)
            gp_all = cpool.tile([BATCH, T, COLS], bf16)

            # ---- persistent state tiles ----
            def state(val=0.0):
                t_ = spool.tile([BATCH, COLS], f32, tag=f"st{state.i}")
                state.i += 1
                nc.vector.memset(t_[:], val)
                return t_
            state.i = 0

            U = state(-65.0)
            ref = state()
            x0, x1, x2 = state(), state(), state()
            g0, g1, g2 = state(), state(), state()
            xF, gF = state(), state()
            s_sb = state()
            m_t = state()
            tq = state()
            rb = state()
            inner = state()
            gp_t = state()

            stt = nc.vector.scalar_tensor_tensor
            gstt = nc.gpsimd.scalar_tensor_tensor

            # ---- FF precompute: unpack spikes, project, dual-exp scan ----
            for tb in range(T // TBLK):
                unp = upool.tile([128, K_FF, TBLK, BATCH], bf16, tag="unp")
                ubit = upool.tile([128, K_FF, TBLK], u8, tag="ubit")
                for b in range(BATCH):
                    w, j = b // 8, b % 8
                    nc.vector.tensor_scalar(
                        ubit[:], pk_sb[:, :, tb * TBLK:(tb + 1) * TBLK, w],
                        1 << j, None, op0=op.bitwise_and)
                    nc.vector.tensor_scalar(
                        unp[:, :, :, b], ubit[:], 0, None, op0=op.is_gt)
                for tl in range(TBLK):
                    t = tb * TBLK + tl
                    pff = pff_pool.tile([BATCH, COLS], f32)
                    for k in range(K_FF):
                        nc.tensor.matmul(pff[:], unp[:, k, tl, :], wf_sb[:, k, :],
                                         start=(k == 0), stop=(k == K_FF - 1))
                    stt(xF[:], xF[:], ARF, pff[:], op.mult, op.add)
                    stt(gF[:], gF[:], ADF, xF[:], op.mult, op.add)
                    # Gp = gF + 10 (folds the g_l leak term into gtot)
                    nc.gpsimd.tensor_scalar(gp_all[:, t, :], gF[:], 10.0, None,
                                            op0=op.add)

            sT_cur = st_pool.tile([128, K_REC, BATCH], bf16)
            nc.vector.memset(sT_cur[:], 0.0)

            for t in range(T):
                tl = t % OBLK
                if tl == 0:
                    s_stage = o_pool.tile([BATCH, OBLK, COLS], u8, tag="sst")
                    u_stage = o_pool.tile([BATCH, OBLK, COLS], f8, tag="ust")
                # recurrent matmul: gathered spike vector stationary (bf16)
                pinp = pin_pool.tile([BATCH, 2 * COLS], f32)
                if "nomm" in abl:
                    nc.vector.memset(pinp[:], 0.0)
                for k in range(0 if "nomm" in abl else K_REC):
                    nc.tensor.matmul(pinp[:], sT_cur[:, k, :], w_sb[:, k, :],
                                     start=(k == 0), stop=(k == K_REC - 1))

                # refractory bookkeeping + Gp fetch: no dep on this step's
                # matmul, runs on Pool/Scalar during the matmuls.
                nc.gpsimd.tensor_scalar(m_t[:], ref[:], 0.0, None, op0=op.is_gt)
                nc.gpsimd.tensor_scalar(ref[:], ref[:], -1.0, 0.0, op0=op.add, op1=op.max)
                nc.scalar.copy(gp_t[:], gp_all[:, t, :])

                # dual-exponential conductance states
                stt(x0[:], x0[:], AR[0], pinp[:, 0:COLS], op.mult, op.add)
                stt(x1[:], x1[:], AR[1], pinp[:, 0:COLS], op.mult, op.add)
                stt(x2[:], x2[:], AR[2], pinp[:, COLS:2 * COLS], op.mult, op.add)
                stt(g0[:], g0[:], AD[0], x0[:], op.mult, op.add)
                stt(g1[:], g1[:], AD[1], x1[:], op.mult, op.add)
                stt(g2[:], g2[:], AD[2], x2[:], op.mult, op.add)

                # Q3 = gtot + 10 = g0 + 0.5*g1 + g2 + Gp[t]
                stt(tq[:], g1[:], 0.5, g0[:], op.mult, op.add)
                nc.vector.tensor_tensor(tq[:], tq[:], g2[:], op.add)
                nc.vector.tensor_tensor(tq[:], tq[:], gp_t[:], op.add)
                # Rb = gE - 650 = -70*g2 - 650
                stt(rb[:], g2[:], -70.0, c650[:], op.mult, op.subtract)
                # U += lc * (Rb - Q3*U)
                nc.vector.tensor_tensor(inner[:], tq[:], U[:], op.mult)
                nc.vector.tensor_tensor(inner[:], rb[:], inner[:], op.subtract)
                nc.vector.tensor_tensor(inner[:], inner[:], lc_t[:], op.mult)
                nc.vector.tensor_tensor(U[:], U[:], inner[:], op.add)
                # refractory clamp, spike, reset
                nc.vector.copy_predicated(U[:], m_t[:].bitcast(mybir.dt.int32), neg65[:])
                nc.vector.tensor_scalar(s_sb[:], U[:], -50.0, None, op0=op.is_ge)
                s_mask = s_sb[:].bitcast(mybir.dt.int32)
                nc.vector.copy_predicated(U[:], s_mask, neg65[:])
                nc.vector.copy_predicated(ref[:], s_mask, rs_t[:])

                if t < T - 1:
                    # transpose own spike slice to [neuron, batch] and gather
                    ptr = ptr_pool.tile([128, 2 * BATCH], f32)
                    nc.tensor.transpose(ptr[0:128, 0:BATCH], s_sb[:, 0:128], ident[:])
                    nc.tensor.transpose(ptr[0:64, BATCH:2 * BATCH],
                                        s_sb[:, 128:COLS], ident[:])
                    sp_st = st_pool.tile([128, 2 * BATCH], bf16, tag="spst")
                    nc.scalar.copy(sp_st[:], ptr[:])
                    agi = agi_pool.tile([COLS, BATCH], bf16)
                    nc.sync.dma_start(agi[0:128, :], sp_st[0:128, 0:BATCH])
                    nc.sync.dma_start(agi[128:COLS, :], sp_st[0:64, BATCH:2 * BATCH])
                    ago = ago_pool.tile([N_NEURONS, BATCH], bf16)
                    if "nocc" in abl:
                        nc.sync.dma_start(ago.opt()[0:COLS], agi.opt())
                    else:
                        nc.gpsimd.collective_compute(
                            "AllGather",
                            op.bypass,
                            replica_groups=[list(range(N_CORES))],
                            ins=[agi.opt()],
                            outs=[ago.opt()],
                        )
                    sT_cur = st_pool.tile([128, K_REC, BATCH], bf16)
                    ago_v = ago.opt().rearrange("(k p) b -> p k b", p=128)
                    # 12 separate DMAs spread across HWDGE queues: each moves a
                    # contiguous k-tile, cutting the serial gather-return
                    # latency vs one strided transfer.
                    for k in range(K_REC):
                        nc.sync.dma_start(sT_cur[:, k, :], ago_v[:, k, :])

                # stage outputs; flush every OBLK steps
                nc.scalar.copy(s_stage[:, tl, :], s_sb[:])
                nc.scalar.activation(u_stage[:, tl, :], U[:],
                                     mybir.ActivationFunctionType.Copy, bias=65.0)
                if tl == OBLK - 1 and "nodma" not in abl:
                    t0 = t - OBLK + 1
                    nc.sync.dma_start(out_s[:, t0:t0 + OBLK, :], s_stage[:])
                    nc.sync.dma_start(out_u[:, t0:t0 + OBLK, :], u_stage[:])

    nc.compile()
    return nc


def _prep_inputs(input_spikes, weights, weights_FF, scaling_factors,
                 scaling_factors_FF, cell_type_indices, cell_type_indices_FF, T):
    import ml_dtypes
    bf16 = ml_dtypes.bfloat16

    ct = np.asarray(cell_type_indices).astype(np.int64)
    sf = np.asarray(scaling_factors, np.float32)[ct[:, None], ct[None, :]]
    W = np.asarray(weights, np.float32) * sf
    mask_e = (ct == 0).astype(np.float32)[:, None]
    W_e = W * mask_e
    W_i = W * (1.0 - mask_e)
    ctF = np.asarray(cell_type_indices_FF).astype(np.int64)
    sfF = np.asarray(scaling_factors_FF, np.float32)[ctF[:, None], ct[None, :]]
    WF = np.asarray(weights_FF, np.float32) * sfF

    tau_mem = CELL_TAU_MEM[ct]
    lc = (DT / (tau_mem * 10.0)).astype(np.float32)        # leak_coef per neuron
    rs = (CELL_TAUREF[ct] / DT).astype(np.float32)          # refractory steps

    isp = np.asarray(input_spikes, np.float32)[:, :T, :]
    # pk[k, p, t, w] = packbits over batch of input_spikes[b, t, 128k+p]
    bits = np.ascontiguousarray(isp.transpose(2, 1, 0)) > 0.5   # (768, T, 32)
    pk = np.packbits(bits.reshape(K_FF, 128, T, BATCH), axis=-1,
                     bitorder="little")                          # (6,128,T,4)

    ident = np.eye(BATCH, dtype=np.float32)

    in_maps = []
    for c in range(N_CORES):
        cols = slice(c * COLS, (c + 1) * COLS)
        wcat = np.concatenate([W_e[:, cols], W_i[:, cols]], axis=1)  # (1536, 384)
        w_in = np.ascontiguousarray(wcat.reshape(K_REC, 128, 2 * COLS)).astype(bf16)
        wf_c = np.ascontiguousarray(WF[:, cols].reshape(K_FF, 128, COLS)).astype(bf16)
        lc_c = np.broadcast_to(lc[cols], (BATCH, COLS)).copy()
        rs_c = np.broadcast_to(rs[cols], (BATCH, COLS)).copy()
        in_maps.append({
            "w_in": w_in,
            "wf_in": wf_c,
            "pk_in": pk,
            "lc_in": lc_c,
            "rs_in": rs_c,
            "id_in": ident,
        })
    return in_maps


_NC_CACHE = {}


def run(inputs: dict, T: int = T_STEPS, trace: bool = False):
    from concourse.bass_utils import run_bass_kernel_spmd

    if T not in _NC_CACHE:
        _NC_CACHE[T] = _build(T)
    nc = _NC_CACHE[T]
    in_maps = _prep_inputs(T=T, **inputs)
    res = run_bass_kernel_spmd(
        nc, in_maps, core_ids=list(range(N_CORES)), trace=trace,
    )
    spk = np.concatenate([r["out_s"] for r in res.results], axis=2).astype(np.float32)
    volts = np.concatenate(
        [r["out_u"].astype(np.float32) for r in res.results], axis=2) - 65.0
    return (spk, volts.astype(np.float32)), res


def kernel(**inputs):
    (spk, volts), _ = run(inputs, T=T_STEPS, trace=False)
    return spk, volts


# revision 25
# speedup vs baseline: 4.4059x; 1.2694x over previous
"""Trainium2 Bass kernel for the ConductanceLIFNetwork problem.

Strategy: column-shard the 1536 postsynaptic neurons across 8 cores (192
each), batch (32) replicated; exchange spike slices per step with an
AllGather.  The runtime here is dominated by per-instruction overhead, so
the kernel is built to minimize total instruction count:

- The feedforward conductance gF is input-independent of the recurrent
  state, so it is precomputed before the loop: bitpacked input spikes are
  expanded to bf16, projected through the (leak-coefficient-folded) FF
  weights on the PE, and convolved with the dual-exponential kernel h(d)
  as two 128x128 Toeplitz matmuls per batch row -- no sequential scan.
- The per-step LIF update runs in 16 vector-engine instructions on fused
  state tiles ([x0|x1|x2|g0|g1|g2] as one [32,1152] tile), using a
  stride-0 broadcast view to add the excitatory PSUM slice to both x0 and
  x1 in one op, and reading the precomputed bf16 Gp[t] slice directly as
  a mixed-dtype operand.  The leak coefficient is folded into all weights
  and constants so no per-neuron multiply survives in the loop.
- Outputs are staged in SBUF and flushed every 32 steps as u8 spikes and
  fp8(U+65) voltages; all weights ship as bf16 and spikes as packed bits
  to minimize host I/O, which is the other dominant cost.
"""

import math

import numpy as np

# ---- problem constants (hardcoded; kernel.py must be self-contained) ----
N_NEURONS = 1536
N_INPUTS = 768
BATCH = 32
T_STEPS = 256
N_CORES = 8
COLS = N_NEURONS // N_CORES  # 192 postsynaptic neurons per core
DT = 1.0

CELL_TAU_MEM = np.array([20.0, 10.0], np.float32)
CELL_TAUREF = np.array([2.0, 1.0], np.float32)
# theta=-50, u_reset=e_l=-65, g_l=10 for both cell types
SYN_TAU_RISE = np.array([0.5, 2.0, 0.5], np.float32)
SYN_TAU_DECAY = np.array([2.0, 100.0, 5.0], np.float32)

AR = [float(math.exp(-DT / t)) for t in SYN_TAU_RISE]   # x rise decays
AD = [float(math.exp(-DT / t)) for t in SYN_TAU_DECAY]  # g decay
ARF = float(math.exp(-DT / 0.5))
ADF = float(math.exp(-DT / 2.0))

K_REC = N_NEURONS // 128   # 12 contraction tiles for recurrent matmul
K_FF = N_INPUTS // 128     # 6 contraction tiles for feedforward matmul
OBLK = 8                   # output staging block
HBLK = 128                 # t-block for FF convolution
QBLK = 64                  # t-block for the spike unpack buffer


def _build(T: int):
    import os
    abl = set(os.environ.get("KABL", "").split(","))
    import concourse.bacc as bacc
    import concourse.tile as tile
    import concourse.mybir as mybir

    f32 = mybir.dt.float32
    bf16 = mybir.dt.bfloat16
    u8 = mybir.dt.uint8
    f8 = mybir.dt.float8e4
    i32 = mybir.dt.int32
    op = mybir.AluOpType

    assert T % HBLK == 0 and T // HBLK == 2

    nc = bacc.Bacc(
        "TRN2",
        target_bir_lowering=False,
        debug=False,
        enable_asserts=False,
        num_devices=N_CORES,
    )

    # ---- kernel I/O ----
    w_in = nc.dram_tensor("w_in", [K_REC, 128, 2 * COLS], bf16, kind="ExternalInput").ap()
    wf_in = nc.dram_tensor("wf_in", [K_FF, 128, COLS], bf16, kind="ExternalInput").ap()
    pk_in = nc.dram_tensor("pk_in", [K_FF, 128, T, BATCH // 8], u8, kind="ExternalInput").ap()
    h_in = nc.dram_tensor("h_in", [2, 128, 128], bf16, kind="ExternalInput").ap()
    cst_in = nc.dram_tensor("cst_in", [1, 2 * COLS], f32, kind="ExternalInput").ap()
    rs_in = nc.dram_tensor("rs_in", [BATCH, COLS], f32, kind="ExternalInput").ap()
    id_in = nc.dram_tensor("id_in", [BATCH, BATCH], f32, kind="ExternalInput").ap()
    out_s = nc.dram_tensor("out_s", [BATCH, T // 8, COLS], u8, kind="ExternalOutput").ap()
    out_u = nc.dram_tensor("out_u", [BATCH, T, COLS], f8, kind="ExternalOutput").ap()

    with tile.TileContext(nc) as tc:
        with (
            tc.tile_pool(name="const", bufs=1) as cpool,
            tc.tile_pool(name="state", bufs=1) as spool,
            tc.tile_pool(name="unp", bufs=1) as upool,
            tc.tile_pool(name="st", bufs=2) as st_pool,
            tc.tile_pool(name="stage", bufs=2) as o_pool,
            tc.tile_pool(name="pin", bufs=2, space="PSUM") as pin_pool,
            tc.tile_pool(name="pff", bufs=2, space="PSUM") as pff_pool,
            tc.tile_pool(name="pcv", bufs=1, space="PSUM") as pcv_pool,
            tc.tile_pool(name="ptr", bufs=2, space="PSUM") as ptr_pool,
            tc.tile_pool(name="agi", bufs=2, space="DRAM") as agi_pool,
            tc.tile_pool(name="ago", bufs=2, space="DRAM") as ago_pool,
        ):
            # ---- load constants ----
            w_sb = cpool.tile([128, K_REC, 2 * COLS], bf16)
            nc.sync.dma_start(w_sb[:], w_in.rearrange("k p c -> p k c"))
            wf_sb = cpool.tile([128, K_FF, COLS], bf16)
            nc.sync.dma_start(wf_sb[:], wf_in.rearrange("k p c -> p k c"))
            h_sb = cpool.tile([128, 2, 128], bf16)
            nc.sync.dma_start(h_sb[:], h_in.rearrange("h p t -> p h t"))
            cst = cpool.tile([1, 2 * COLS], f32)
            nc.sync.dma_start(cst[:], cst_in)
            rs_t = cpool.tile([BATCH, COLS], f32)
            nc.sync.dma_start(rs_t[:], rs_in)
            ident = cpool.tile([BATCH, BATCH], f32)
            nc.sync.dma_start(ident[:], id_in)
            neg65 = cpool.tile([BATCH, COLS], f32)
            nc.vector.memset(neg65[:], -65.0)
            # c650lc[b, n] = 650*lc[n]; c10lc128[t, n] = 10*lc[n]
            c650lc = cpool.tile([BATCH, COLS], f32)
            nc.gpsimd.partition_broadcast(c650lc[:], cst[0:1, 0:COLS])
            c10lc = cpool.tile([128, COLS], f32)
            nc.gpsimd.partition_broadcast(c10lc[:], cst[0:1, COLS:2 * COLS])

            # decay multipliers for the fused [x0|x1|x2|g0|g1|g2] tile
            A6 = cpool.tile([BATCH, 6, COLS], f32)
            for i, a in enumerate(AR + AD):
                nc.vector.memset(A6[:, i, :], a)

            # Gp[b, t, n] = lc[n] * (gF[t, b, n] + 10)
            gp_all = cpool.tile([BATCH, T, COLS], bf16)

            # ---- persistent state tiles ----
            B6 = spool.tile([BATCH, 6, COLS], f32, tag="B6")
            nc.vector.memset(B6[:], 0.0)
            U = spool.tile([BATCH, COLS], f32, tag="U")
            nc.vector.memset(U[:], -65.0)
            ref = spool.tile([BATCH, COLS], f32, tag="ref")
            nc.vector.memset(ref[:], 0.0)
            s_sb = spool.tile([BATCH, COLS], f32, tag="s")
            tq = spool.tile([BATCH, COLS], f32, tag="tq")
            rb = spool.tile([BATCH, COLS], f32, tag="rb")
            dd = spool.tile([BATCH, COLS], f32, tag="dd")
            pk_acc = spool.tile([BATCH, COLS], f32, tag="pka")

            x0 = B6[:, 0, :]
            x1 = B6[:, 1, :]
            x2 = B6[:, 2, :]
            g0 = B6[:, 3, :]
            g1 = B6[:, 4, :]
            g2 = B6[:, 5, :]

            stt = nc.vector.scalar_tensor_tensor
            tt = nc.vector.tensor_tensor

            # ---- FF precompute ----
            P_sb = cpool.tile([128, 2, BATCH, COLS], bf16)
            for q in range(T // QBLK):
                pk_sb = upool.tile([128, K_FF, QBLK, BATCH // 8], u8, tag="pk")
                nc.sync.dma_start(
                    pk_sb[:],
                    pk_in[:, :, q * QBLK:(q + 1) * QBLK, :].rearrange(
                        "k p t w -> p k t w"))
                unp = upool.tile([128, K_FF, QBLK, BATCH], bf16, tag="unp")
                ubit = upool.tile([128, K_FF, QBLK], u8, tag="ubit")
                for b in range(BATCH):
                    w, j = b // 8, b % 8
                    nc.vector.tensor_scalar(
                        ubit[:], pk_sb[:, :, :, w], 1 << j, None,
                        op0=op.bitwise_and)
                    nc.vector.tensor_scalar(
                        unp[:, :, :, b], ubit[:], 0, None, op0=op.is_gt)
                blk, half = q // 2, q % 2
                psl = slice(half * QBLK, (half + 1) * QBLK)
                for b in range(BATCH):
                    pff = pff_pool.tile([QBLK, COLS], f32)
                    for k in range(K_FF):
                        nc.tensor.matmul(pff[:], unp[:, k, :, b], wf_sb[:, k, :],
                                         start=(k == 0), stop=(k == K_FF - 1))
                    nc.scalar.copy(P_sb[psl, blk, b, :], pff[:])
            # causal dual-exponential convolution via Toeplitz matmuls
            for b in range(BATCH):
                gps = pcv_pool.tile([128, COLS], f32, tag="gps0")
                nc.tensor.matmul(gps[:], h_sb[:, 0, :], P_sb[:, 0, b, :],
                                 start=True, stop=True)
                gpt = st_pool.tile([128, COLS], bf16, tag="gpt")
                tt(gpt[:], gps[:], c10lc[:], op.add)
                nc.sync.dma_start(gp_all[b:b + 1, 0:HBLK, :], gpt[:])
                gps2 = pcv_pool.tile([128, COLS], f32, tag="gps1")
                nc.tensor.matmul(gps2[:], h_sb[:, 1, :], P_sb[:, 0, b, :],
                                 start=True, stop=False)
                nc.tensor.matmul(gps2[:], h_sb[:, 0, :], P_sb[:, 1, b, :],
                                 start=False, stop=True)
                gpt2 = st_pool.tile([128, COLS], bf16, tag="gpt")
                tt(gpt2[:], gps2[:], c10lc[:], op.add)
                nc.sync.dma_start(gp_all[b:b + 1, HBLK:T, :], gpt2[:])

            # ---- main loop ----
            sT_cur = st_pool.tile([128, K_REC, BATCH], bf16, tag="sT")
            nc.vector.memset(sT_cur[:], 0.0)

            for t in range(T):
                tl = t % OBLK
                if tl == 0:
                    s_stage = o_pool.tile([BATCH, OBLK // 8, COLS], u8, tag="sst")
                    u_stage = o_pool.tile([BATCH, OBLK, COLS], f8, tag="ust")

                # recurrent matmul: gathered spike vector stationary (bf16)
                pinp = pin_pool.tile([BATCH, 2 * COLS], f32)
                if "nomm" in abl:
                    nc.vector.memset(pinp[:], 0.0)
                for k in range(0 if "nomm" in abl else K_REC):
                    nc.tensor.matmul(pinp[:], sT_cur[:, k, :], w_sb[:, k, :],
                                     start=(k == 0), stop=(k == K_REC - 1))

                # x *= ar, g *= ad (one fused op over all six states; runs on
                # Pool in parallel with the matmul)
                nc.gpsimd.tensor_tensor(B6[:], B6[:], A6[:], op.mult)
                # x0 += pe, x1 += pe (stride-0 broadcast of the PSUM slice)
                pe2 = pinp[:, 0:COLS].unsqueeze(1).broadcast_to((BATCH, 2, COLS))
                tt(B6[:, 0:2, :], B6[:, 0:2, :], pe2, op.add)
                tt(x2, x2, pinp[:, COLS:2 * COLS], op.add)
                # g += x
                tt(B6[:, 3:6, :], B6[:, 3:6, :], B6[:, 0:3, :], op.add)

                # Rb = lc*(gE - 650) = -70*g2 - 650lc
                stt(rb[:], g2, -70.0, c650lc[:], op.mult, op.subtract)
                # Q3 = lc*(gtot + 10) = g0 + 0.5*g1 + g2 + Gp[t]
                stt(tq[:], g1, 0.5, g0, op.mult, op.add)
                tt(tq[:], tq[:], g2, op.add)
                tt(tq[:], tq[:], gp_all[:, t, :], op.add)
                # U += Rb - Q3*U
                tt(dd[:], tq[:], U[:], op.mult)
                stt(dd[:], dd[:], -1.0, rb[:], op.mult, op.add)
                tt(U[:], U[:], dd[:], op.add)
                # refractory clamp (ref>0 pre-decrement), spike, reset
                nc.vector.copy_predicated(U[:], ref[:].bitcast(i32), neg65[:])
                nc.gpsimd.tensor_scalar(ref[:], ref[:], -1.0, 0.0,
                                        op0=op.add, op1=op.max)
                nc.vector.tensor_scalar(s_sb[:], U[:], -50.0, None, op0=op.is_ge)
                s_mask = s_sb[:].bitcast(i32)
                nc.vector.copy_predicated(U[:], s_mask, neg65[:])
                nc.vector.copy_predicated(ref[:], s_mask, rs_t[:])

                if t < T - 1:
                    # transpose own spike slice to [neuron, batch] and gather
                    ptr = ptr_pool.tile([96, 2 * BATCH], f32)
                    nc.tensor.transpose(ptr[0:96, 0:BATCH], s_sb[:, 0:96], ident[:])
                    nc.tensor.transpose(ptr[0:96, BATCH:2 * BATCH],
                                        s_sb[:, 96:COLS], ident[:])
                    sp_st = st_pool.tile([96, 2, BATCH], bf16, tag="spst")
                    nc.scalar.copy(sp_st[:], ptr[:].rearrange("p (c b) -> p c b", c=2))
                    agi = agi_pool.tile([COLS, BATCH], bf16)
                    nc.sync.dma_start(
                        agi.opt().rearrange("(c p) b -> p c b", p=96), sp_st[:])
                    ago = ago_pool.tile([N_NEURONS, BATCH], bf16)
                    if "nocc" in abl:
                        nc.sync.dma_start(ago.opt()[0:COLS], agi.opt())
                    else:
                        nc.gpsimd.collective_compute(
                            "AllGather",
                            op.bypass,
                            replica_groups=[list(range(N_CORES))],
                            ins=[agi.opt()],
                            outs=[ago.opt()],
                        )
                    sT_cur = st_pool.tile([128, K_REC, BATCH], bf16, tag="sT")
                    ago_v = ago.opt().rearrange("(k p) b -> p k b", p=128)
                    nc.sync.dma_start(sT_cur[:], ago_v)

                # stage outputs: bit-pack spikes (8 steps/byte), fp8 voltages;
                # flush every OBLK steps
                j = t % 8
                if j == 0:
                    nc.vector.tensor_scalar(pk_acc[:], s_sb[:], 1.0, None,
                                            op0=op.mult)
                else:
                    stt(pk_acc[:], s_sb[:], float(1 << j), pk_acc[:],
                        op.mult, op.add)
                if j == 7:
                    nc.scalar.copy(s_stage[:, tl // 8, :], pk_acc[:])
                nc.scalar.activation(u_stage[:, tl, :], U[:],
                                     mybir.ActivationFunctionType.Copy, bias=65.0)
                if tl == OBLK - 1 and "nodma" not in abl:
                    t0 = t - OBLK + 1
                    nc.sync.dma_start(out_s[:, t0 // 8:(t0 + OBLK) // 8, :],
                                      s_stage[:])
                    nc.sync.dma_start(out_u[:, t0:t0 + OBLK, :], u_stage[:])

    nc.compile()
    return nc


def _prep_inputs(input_spikes, weights, weights_FF, scaling_factors,
                 scaling_factors_FF, cell_type_indices, cell_type_indices_FF, T):
    import ml_dtypes
    bf16 = ml_dtypes.bfloat16

    ct = np.asarray(cell_type_indices).astype(np.int64)
    sf = np.asarray(scaling_factors, np.float32)[ct[:, None], ct[None, :]]
    W = np.asarray(weights, np.float32) * sf
    mask_e = (ct == 0).astype(np.float32)[:, None]
    W_e = W * mask_e
    W_i = W * (1.0 - mask_e)
    ctF = np.asarray(cell_type_indices_FF).astype(np.int64)
    sfF = np.asarray(scaling_factors_FF, np.float32)[ctF[:, None], ct[None, :]]
    WF = np.asarray(weights_FF, np.float32) * sfF

    tau_mem = CELL_TAU_MEM[ct]
    lc = (DT / (tau_mem * 10.0)).astype(np.float32)        # leak_coef per neuron
    rs = (CELL_TAUREF[ct] / DT).astype(np.float32)          # refractory steps

    isp = np.asarray(input_spikes, np.float32)[:, :T, :]
    # pk[k, p, t, w] = packbits over batch of input_spikes[b, t, 128k+p]
    bits = np.ascontiguousarray(isp.transpose(2, 1, 0)) > 0.5   # (768, T, 32)
    pk = np.packbits(bits.reshape(K_FF, 128, T, BATCH), axis=-1,
                     bitorder="little")                          # (6,128,T,4)

    # dual-exponential causal kernel h(d) = (ad^(d+1) - ar^(d+1))/(ad - ar)
    d = np.arange(2 * HBLK, dtype=np.float64)
    h = (ADF ** (d + 1) - ARF ** (d + 1)) / (ADF - ARF)
    tt_, tau_ = np.meshgrid(np.arange(HBLK), np.arange(HBLK), indexing="xy")
    # h_in[0][tau, t] = h(t - tau) (lower block), h_in[1][tau, t] = h(128+t-tau)
    dmat = tt_ - tau_
    h0 = np.where(dmat >= 0, h[np.abs(dmat)], 0.0)
    h1 = h[dmat + HBLK]
    h_in = np.stack([h0, h1]).astype(bf16)                       # (2,128,128)

    ident = np.eye(BATCH, dtype=np.float32)

    in_maps = []
    for c in range(N_CORES):
        cols = slice(c * COLS, (c + 1) * COLS)
        lcc = lc[cols]
        wcat = np.concatenate([W_e[:, cols] * lcc, W_i[:, cols] * lcc], axis=1)
        w_in = np.ascontiguousarray(wcat.reshape(K_REC, 128, 2 * COLS)).astype(bf16)
        wf_c = np.ascontiguousarray(
            (WF[:, cols] * lcc).reshape(K_FF, 128, COLS)).astype(bf16)
        cst = np.concatenate([650.0 * lcc, 10.0 * lcc]).astype(
            np.float32).reshape(1, 2 * COLS)
        rs_c = np.broadcast_to(rs[cols], (BATCH, COLS)).copy()
        in_maps.append({
            "w_in": w_in,
            "wf_in": wf_c,
            "pk_in": pk,
            "h_in": h_in,
            "cst_in": cst,
            "rs_in": rs_c,
            "id_in": ident,
        })
    return in_maps


_NC_CACHE = {}


def run(inputs: dict, T: int = T_STEPS, trace: bool = False):
    from concourse.bass_utils import run_bass_kernel_spmd

    if T not in _NC_CACHE:
        _NC_CACHE[T] = _build(T)
    nc = _NC_CACHE[T]
    in_maps = _prep_inputs(T=T, **inputs)
    res = run_bass_kernel_spmd(
        nc, in_maps, core_ids=list(range(N_CORES)), trace=trace,
    )
    spk_pk = np.concatenate([r["out_s"] for r in res.results], axis=2)
    spk = np.unpackbits(spk_pk, axis=1, bitorder="little").astype(np.float32)
    volts = np.concatenate(
        [r["out_u"].astype(np.float32) for r in res.results], axis=2) - 65.0
    return (spk, volts.astype(np.float32)), res


def kernel(**inputs):
    (spk, volts), _ = run(inputs, T=T_STEPS, trace=False)
    return spk, volts


# revision 32
# speedup vs baseline: 4.9326x; 1.1195x over previous
"""Trainium2 Bass kernel for the ConductanceLIFNetwork problem.

Strategy: column-shard the 1536 postsynaptic neurons across 8 cores (192
each), batch (32) replicated; exchange spike slices per step with an
AllGather.  The runtime here is dominated by per-instruction overhead, so
the kernel is built to minimize total instruction count:

- The feedforward conductance gF is input-independent of the recurrent
  state, so it is precomputed before the loop: bitpacked input spikes are
  expanded to bf16, projected through the (leak-coefficient-folded) FF
  weights on the PE, and convolved with the dual-exponential kernel h(d)
  as two 128x128 Toeplitz matmuls per batch row -- no sequential scan.
- The per-step LIF update runs in 16 vector-engine instructions on fused
  state tiles ([x0|x1|x2|g0|g1|g2] as one [32,1152] tile), using a
  stride-0 broadcast view to add the excitatory PSUM slice to both x0 and
  x1 in one op, and reading the precomputed bf16 Gp[t] slice directly as
  a mixed-dtype operand.  The leak coefficient is folded into all weights
  and constants so no per-neuron multiply survives in the loop.
- Outputs are staged in SBUF and flushed every 32 steps as u8 spikes and
  fp8(U+65) voltages; all weights ship as bf16 and spikes as packed bits
  to minimize host I/O, which is the other dominant cost.
"""

import math

import numpy as np

# ---- problem constants (hardcoded; kernel.py must be self-contained) ----
N_NEURONS = 1536
N_INPUTS = 768
BATCH = 32
T_STEPS = 256
N_CORES = 8
COLS = N_NEURONS // N_CORES  # 192 postsynaptic neurons per core
DT = 1.0

CELL_TAU_MEM = np.array([20.0, 10.0], np.float32)
CELL_TAUREF = np.array([2.0, 1.0], np.float32)
# theta=-50, u_reset=e_l=-65, g_l=10 for both cell types
SYN_TAU_RISE = np.array([0.5, 2.0, 0.5], np.float32)
SYN_TAU_DECAY = np.array([2.0, 100.0, 5.0], np.float32)

AR = [float(math.exp(-DT / t)) for t in SYN_TAU_RISE]   # x rise decays
AD = [float(math.exp(-DT / t)) for t in SYN_TAU_DECAY]  # g decay
ARF = float(math.exp(-DT / 0.5))
ADF = float(math.exp(-DT / 2.0))

K_REC = N_NEURONS // 128   # 12 contraction tiles for recurrent matmul
K_FF = N_INPUTS // 128     # 6 contraction tiles for feedforward matmul
OBLK = 8                   # output staging block
HBLK = 128                 # t-block for FF convolution
QBLK = 64                  # t-block for the spike unpack buffer


def _build(T: int):
    import os
    abl = set(os.environ.get("KABL", "").split(","))
    import concourse.bacc as bacc
    import concourse.tile as tile
    import concourse.mybir as mybir

    f32 = mybir.dt.float32
    bf16 = mybir.dt.bfloat16
    u8 = mybir.dt.uint8
    f8 = mybir.dt.float8e4
    i32 = mybir.dt.int32
    op = mybir.AluOpType

    assert T % HBLK == 0 and T // HBLK == 2

    nc = bacc.Bacc(
        "TRN2",
        target_bir_lowering=False,
        debug=False,
        enable_asserts=False,
        num_devices=N_CORES,
    )

    # ---- kernel I/O ----
    w_in = nc.dram_tensor("w_in", [K_REC, 128, 2 * COLS], f8, kind="ExternalInput").ap()
    wf_in = nc.dram_tensor("wf_in", [K_FF, 128, COLS], bf16, kind="ExternalInput").ap()
    pk_in = nc.dram_tensor("pk_in", [K_FF, 128, T, BATCH // 8], u8, kind="ExternalInput").ap()
    h_in = nc.dram_tensor("h_in", [2, 128, 128], bf16, kind="ExternalInput").ap()
    cst_in = nc.dram_tensor("cst_in", [1, 2 * COLS], f32, kind="ExternalInput").ap()
    rs_in = nc.dram_tensor("rs_in", [BATCH, COLS], f32, kind="ExternalInput").ap()
    id_in = nc.dram_tensor("id_in", [BATCH, BATCH], f32, kind="ExternalInput").ap()
    out_s = nc.dram_tensor("out_s", [BATCH, T // 8, COLS], u8, kind="ExternalOutput").ap()
    out_u = nc.dram_tensor("out_u", [BATCH, T, COLS], f8, kind="ExternalOutput").ap()

    with tile.TileContext(nc) as tc:
        with (
            tc.tile_pool(name="const", bufs=1) as cpool,
            tc.tile_pool(name="state", bufs=1) as spool,
            tc.tile_pool(name="unp", bufs=1) as upool,
            tc.tile_pool(name="st", bufs=2) as st_pool,
            tc.tile_pool(name="stage", bufs=2) as o_pool,
            tc.tile_pool(name="pin", bufs=2, space="PSUM") as pin_pool,
            tc.tile_pool(name="pff", bufs=2, space="PSUM") as pff_pool,
            tc.tile_pool(name="pcv", bufs=1, space="PSUM") as pcv_pool,
            tc.tile_pool(name="ptr", bufs=2, space="PSUM") as ptr_pool,
            tc.tile_pool(name="agi", bufs=2, space="DRAM") as agi_pool,
            tc.tile_pool(name="ago", bufs=2, space="DRAM") as ago_pool,
        ):
            # ---- load constants ----
            w_sb = cpool.tile([128, K_REC, 2 * COLS], f8)
            nc.sync.dma_start(w_sb[:], w_in.rearrange("k p c -> p k c"))
            wf_sb = cpool.tile([128, K_FF, COLS], bf16)
            nc.sync.dma_start(wf_sb[:], wf_in.rearrange("k p c -> p k c"))
            h_sb = cpool.tile([128, 2, 128], bf16)
            nc.sync.dma_start(h_sb[:], h_in.rearrange("h p t -> p h t"))
            cst = cpool.tile([1, 2 * COLS], f32)
            nc.sync.dma_start(cst[:], cst_in)
            rs_t = cpool.tile([BATCH, COLS], f32)
            nc.sync.dma_start(rs_t[:], rs_in)
            ident = cpool.tile([BATCH, BATCH], f32)
            nc.sync.dma_start(ident[:], id_in)
            neg65 = cpool.tile([BATCH, COLS], f32)
            nc.vector.memset(neg65[:], -65.0)
            # c650lc[b, n] = 650*lc[n]; c10lc128[t, n] = 10*lc[n]
            c650lc = cpool.tile([BATCH, COLS], f32)
            nc.gpsimd.partition_broadcast(c650lc[:], cst[0:1, 0:COLS])
            c10lc = cpool.tile([128, COLS], f32)
            nc.gpsimd.partition_broadcast(c10lc[:], cst[0:1, COLS:2 * COLS])

            # decay multipliers for the fused [x0|x1|x2|g0|g1|g2] tile
            A6 = cpool.tile([BATCH, 6, COLS], f32)
            for i, a in enumerate(AR + AD):
                nc.vector.memset(A6[:, i, :], a)

            # Gp[b, t, n] = lc[n] * (gF[t, b, n] + 10)
            gp_all = cpool.tile([BATCH, T, COLS], bf16)

            # ---- persistent state tiles ----
            B6 = spool.tile([BATCH, 6, COLS], f32, tag="B6")
            nc.vector.memset(B6[:], 0.0)
            U = spool.tile([BATCH, COLS], f32, tag="U")
            nc.vector.memset(U[:], -65.0)
            ref = spool.tile([BATCH, COLS], f32, tag="ref")
            nc.vector.memset(ref[:], 0.0)
            s_sb = spool.tile([BATCH, COLS], f32, tag="s")
            tq = spool.tile([BATCH, COLS], f32, tag="tq")
            rb = spool.tile([BATCH, COLS], f32, tag="rb")
            dd = spool.tile([BATCH, COLS], f32, tag="dd")
            pk_acc = spool.tile([BATCH, COLS], f32, tag="pka")

            x0 = B6[:, 0, :]
            x1 = B6[:, 1, :]
            x2 = B6[:, 2, :]
            g0 = B6[:, 3, :]
            g1 = B6[:, 4, :]
            g2 = B6[:, 5, :]

            stt = nc.vector.scalar_tensor_tensor
            tt = nc.vector.tensor_tensor

            # ---- FF precompute ----
            P_sb = cpool.tile([128, 2, BATCH, COLS], bf16)
            for q in range(T // QBLK):
                pk_sb = upool.tile([128, K_FF, QBLK, BATCH // 8], u8, tag="pk")
                nc.sync.dma_start(
                    pk_sb[:],
                    pk_in[:, :, q * QBLK:(q + 1) * QBLK, :].rearrange(
                        "k p t w -> p k t w"))
                unp = upool.tile([128, K_FF, QBLK, BATCH], bf16, tag="unp")
                ubit = upool.tile([128, K_FF, QBLK], u8, tag="ubit")
                for b in range(BATCH):
                    w, j = b // 8, b % 8
                    nc.vector.tensor_scalar(
                        ubit[:], pk_sb[:, :, :, w], 1 << j, None,
                        op0=op.bitwise_and)
                    nc.vector.tensor_scalar(
                        unp[:, :, :, b], ubit[:], 0, None, op0=op.is_gt)
                blk, half = q // 2, q % 2
                psl = slice(half * QBLK, (half + 1) * QBLK)
                for b in range(BATCH):
                    pff = pff_pool.tile([QBLK, COLS], f32)
                    for k in range(K_FF):
                        nc.tensor.matmul(pff[:], unp[:, k, :, b], wf_sb[:, k, :],
                                         start=(k == 0), stop=(k == K_FF - 1))
                    nc.scalar.copy(P_sb[psl, blk, b, :], pff[:])
            # causal dual-exponential convolution via Toeplitz matmuls
            for b in range(BATCH):
                gps = pcv_pool.tile([128, COLS], f32, tag="gps0")
                nc.tensor.matmul(gps[:], h_sb[:, 0, :], P_sb[:, 0, b, :],
                                 start=True, stop=True)
                gpt = st_pool.tile([128, COLS], bf16, tag="gpt")
                tt(gpt[:], gps[:], c10lc[:], op.add)
                nc.sync.dma_start(gp_all[b:b + 1, 0:HBLK, :], gpt[:])
                gps2 = pcv_pool.tile([128, COLS], f32, tag="gps1")
                nc.tensor.matmul(gps2[:], h_sb[:, 1, :], P_sb[:, 0, b, :],
                                 start=True, stop=False)
                nc.tensor.matmul(gps2[:], h_sb[:, 0, :], P_sb[:, 1, b, :],
                                 start=False, stop=True)
                gpt2 = st_pool.tile([128, COLS], bf16, tag="gpt")
                tt(gpt2[:], gps2[:], c10lc[:], op.add)
                nc.sync.dma_start(gp_all[b:b + 1, HBLK:T, :], gpt2[:])

            # ---- main loop ----
            sT_cur = st_pool.tile([128, K_REC, BATCH], f8, tag="sT")
            nc.vector.memset(sT_cur[:], 0.0)

            for t in range(T):
                tl = t % OBLK
                if tl == 0:
                    s_stage = o_pool.tile([BATCH, OBLK // 8, COLS], u8, tag="sst")
                    u_stage = o_pool.tile([BATCH, OBLK, COLS], f8, tag="ust")

                # recurrent matmul: gathered spike vector stationary (bf16)
                # fp8 DoubleRow: each instruction contracts 256 presynaptic
                # rows (adjacent k-tile pairs), halving the matmul count.
                # Spikes are stored as 2^-9 (exact in fp8) and the weights
                # pre-scaled by 2^9 host-side, so products are exact.
                pinp = pin_pool.tile([BATCH, 2 * COLS], f32)
                if "nomm" in abl:
                    nc.vector.memset(pinp[:], 0.0)
                for k in range(0 if "nomm" in abl else K_REC // 2):
                    nc.tensor.matmul(pinp[:], sT_cur[:, 2 * k:2 * k + 2, :],
                                     w_sb[:, 2 * k:2 * k + 2, :],
                                     start=(k == 0), stop=(k == K_REC // 2 - 1),
                                     perf_mode=mybir.MatmulPerfMode.DoubleRow)

                # x *= ar, g *= ad (one fused op over all six states; runs on
                # Pool in parallel with the matmul)
                nc.gpsimd.tensor_tensor(B6[:], B6[:], A6[:], op.mult)
                # x0 += pe, x1 += pe (stride-0 broadcast of the PSUM slice)
                pe2 = pinp[:, 0:COLS].unsqueeze(1).broadcast_to((BATCH, 2, COLS))
                tt(B6[:, 0:2, :], B6[:, 0:2, :], pe2, op.add)
                tt(x2, x2, pinp[:, COLS:2 * COLS], op.add)
                # g += x
                tt(B6[:, 3:6, :], B6[:, 3:6, :], B6[:, 0:3, :], op.add)

                # Rb = lc*(gE - 650) = -70*g2 - 650lc
                stt(rb[:], g2, -70.0, c650lc[:], op.mult, op.subtract)
                # Q3 = lc*(gtot + 10) = g0 + 0.5*g1 + g2 + Gp[t]
                stt(tq[:], g1, 0.5, g0, op.mult, op.add)
                tt(tq[:], tq[:], g2, op.add)
                tt(tq[:], tq[:], gp_all[:, t, :], op.add)
                # U += Rb - Q3*U
                tt(dd[:], tq[:], U[:], op.mult)
                stt(dd[:], dd[:], -1.0, rb[:], op.mult, op.add)
                tt(U[:], U[:], dd[:], op.add)
                # refractory clamp (ref>0 pre-decrement), spike, reset
                nc.vector.copy_predicated(U[:], ref[:].bitcast(i32), neg65[:])
                nc.gpsimd.tensor_scalar(ref[:], ref[:], -1.0, 0.0,
                                        op0=op.add, op1=op.max)
                nc.vector.tensor_scalar(s_sb[:], U[:], -50.0, None, op0=op.is_ge)
                s_mask = s_sb[:].bitcast(i32)
                nc.vector.copy_predicated(U[:], s_mask, neg65[:])
                nc.vector.copy_predicated(ref[:], s_mask, rs_t[:])

                if t < T - 1:
                    # transpose own spike slice to [neuron, batch] and gather
                    ptr = ptr_pool.tile([96, 2 * BATCH], f32)
                    nc.tensor.transpose(ptr[0:96, 0:BATCH], s_sb[:, 0:96], ident[:])
                    nc.tensor.transpose(ptr[0:96, BATCH:2 * BATCH],
                                        s_sb[:, 96:COLS], ident[:])
                    sp_st = st_pool.tile([96, 2, BATCH], f8, tag="spst")
                    nc.scalar.activation(
                        sp_st[:], ptr[:].rearrange("p (c b) -> p c b", c=2),
                        mybir.ActivationFunctionType.Copy, scale=1.0 / 512.0)
                    agi = agi_pool.tile([COLS, BATCH], f8)
                    nc.sync.dma_start(
                        agi.opt().rearrange("(c p) b -> p c b", p=96), sp_st[:])
                    ago = ago_pool.tile([N_NEURONS, BATCH], f8)
                    if "nocc" in abl:
                        nc.sync.dma_start(ago.opt()[0:COLS], agi.opt())
                    else:
                        nc.gpsimd.collective_compute(
                            "AllGather",
                            op.bypass,
                            replica_groups=[list(range(N_CORES))],
                            ins=[agi.opt()],
                            outs=[ago.opt()],
                        )
                    sT_cur = st_pool.tile([128, K_REC, BATCH], f8, tag="sT")
                    ago_v = ago.opt().rearrange("(k p) b -> p k b", p=128)
                    nc.sync.dma_start(sT_cur[:], ago_v)

                # stage outputs: bit-pack spikes (8 steps/byte), fp8 voltages;
                # flush every OBLK steps
                j = t % 8
                if j == 0:
                    nc.vector.tensor_scalar(pk_acc[:], s_sb[:], 1.0, None,
                                            op0=op.mult)
                else:
                    stt(pk_acc[:], s_sb[:], float(1 << j), pk_acc[:],
                        op.mult, op.add)
                if j == 7:
                    nc.scalar.copy(s_stage[:, tl // 8, :], pk_acc[:])
                nc.scalar.activation(u_stage[:, tl, :], U[:],
                                     mybir.ActivationFunctionType.Copy, bias=65.0)
                if tl == OBLK - 1 and "nodma" not in abl:
                    t0 = t - OBLK + 1
                    nc.sync.dma_start(out_s[:, t0 // 8:(t0 + OBLK) // 8, :],
                                      s_stage[:])
                    nc.sync.dma_start(out_u[:, t0:t0 + OBLK, :], u_stage[:])

    nc.compile()
    return nc


def _prep_inputs(input_spikes, weights, weights_FF, scaling_factors,
                 scaling_factors_FF, cell_type_indices, cell_type_indices_FF, T):
    import ml_dtypes
    bf16 = ml_dtypes.bfloat16

    ct = np.asarray(cell_type_indices).astype(np.int64)
    sf = np.asarray(scaling_factors, np.float32)[ct[:, None], ct[None, :]]
    W = np.asarray(weights, np.float32) * sf
    mask_e = (ct == 0).astype(np.float32)[:, None]
    W_e = W * mask_e
    W_i = W * (1.0 - mask_e)
    ctF = np.asarray(cell_type_indices_FF).astype(np.int64)
    sfF = np.asarray(scaling_factors_FF, np.float32)[ctF[:, None], ct[None, :]]
    WF = np.asarray(weights_FF, np.float32) * sfF

    tau_mem = CELL_TAU_MEM[ct]
    lc = (DT / (tau_mem * 10.0)).astype(np.float32)        # leak_coef per neuron
    rs = (CELL_TAUREF[ct] / DT).astype(np.float32)          # refractory steps

    isp = np.asarray(input_spikes, np.float32)[:, :T, :]
    # pk[k, p, t, w] = packbits over batch of input_spikes[b, t, 128k+p]
    bits = np.ascontiguousarray(isp.transpose(2, 1, 0)) > 0.5   # (768, T, 32)
    pk = np.packbits(bits.reshape(K_FF, 128, T, BATCH), axis=-1,
                     bitorder="little")                          # (6,128,T,4)

    # dual-exponential causal kernel h(d) = (ad^(d+1) - ar^(d+1))/(ad - ar)
    d = np.arange(2 * HBLK, dtype=np.float64)
    h = (ADF ** (d + 1) - ARF ** (d + 1)) / (ADF - ARF)
    tt_, tau_ = np.meshgrid(np.arange(HBLK), np.arange(HBLK), indexing="xy")
    # h_in[0][tau, t] = h(t - tau) (lower block), h_in[1][tau, t] = h(128+t-tau)
    dmat = tt_ - tau_
    h0 = np.where(dmat >= 0, h[np.abs(dmat)], 0.0)
    h1 = h[dmat + HBLK]
    h_in = np.stack([h0, h1]).astype(bf16)                       # (2,128,128)

    ident = np.eye(BATCH, dtype=np.float32)

    in_maps = []
    for c in range(N_CORES):
        cols = slice(c * COLS, (c + 1) * COLS)
        lcc = lc[cols]
        wcat = np.concatenate([W_e[:, cols] * lcc, W_i[:, cols] * lcc],
                              axis=1) * 512.0
        w_in = np.ascontiguousarray(
            wcat.reshape(K_REC, 128, 2 * COLS)).astype(ml_dtypes.float8_e4m3)
        wf_c = np.ascontiguousarray(
            (WF[:, cols] * lcc).reshape(K_FF, 128, COLS)).astype(bf16)
        cst = np.concatenate([650.0 * lcc, 10.0 * lcc]).astype(
            np.float32).reshape(1, 2 * COLS)
        rs_c = np.broadcast_to(rs[cols], (BATCH, COLS)).copy()
        in_maps.append({
            "w_in": w_in,
            "wf_in": wf_c,
            "pk_in": pk,
            "h_in": h_in,
            "cst_in": cst,
            "rs_in": rs_c,
            "id_in": ident,
        })
    return in_maps


_NC_CACHE = {}


def run(inputs: dict, T: int = T_STEPS, trace: bool = False):
    from concourse.bass_utils import run_bass_kernel_spmd

    if T not in _NC_CACHE:
        _NC_CACHE[T] = _build(T)
    nc = _NC_CACHE[T]
    in_maps = _prep_inputs(T=T, **inputs)
    res = run_bass_kernel_spmd(
        nc, in_maps, core_ids=list(range(N_CORES)), trace=trace,
    )
    spk_pk = np.concatenate([r["out_s"] for r in res.results], axis=2)
    spk = np.unpackbits(spk_pk, axis=1, bitorder="little").astype(np.float32)
    volts = np.concatenate(
        [r["out_u"].astype(np.float32) for r in res.results], axis=2) - 65.0
    return (spk, volts.astype(np.float32)), res


def kernel(**inputs):
    (spk, volts), _ = run(inputs, T=T_STEPS, trace=False)
    return spk, volts
